# revision 13
# baseline (speedup 1.0000x reference)
"""Trainium2 Bass kernel: MHSA with multi-head relative position embedding.

Sharding: data-parallel over batch — 16 batches / 8 cores = 2 batches per core,
each core computes all 8 heads for its 2 batches. No collectives needed.

Math per batch (N=784 tokens, C=512, 8 heads x 64 dim):
  qkv = x @ w_qkv                  (q-columns pre-scaled by 1/8 on host)
  scores_T[k,q] = k_h^T q_h        (head pair packed in one 4-bank PSUM tile;
                                    even/odd head matmuls issued adjacently so
                                    they pack into disjoint PE row groups)
  E = exp(scores_T) * expbias_T    (ONE fused exp on ACT covering both heads'
                                    784-wide rows via a strided AP; bias
                                    multiply split DVE/GPSIMD, bias exp'd host)
  O_T[d,q] = sum_k v_aug[k,d] E[k,q]  with v_aug = [v | 1] -> row 64 = sumexp
  attnT = O_T[0:64] / O_T[64]      (denominator rows DMA-gathered, recip'd,
                                    replicated to 128 partitions with a
                                    stride-0 DMA, DVE mult)
  out = attnT^T stacked over heads @ w_out   (written bf16, host casts f32)

Perf structure vs v1: bias tables loaded ONCE (8 fat DMAs, resident in SBUF
for both batches) instead of per-batch per-tile; PE warmed with dummy matmuls
during the initial DMA wait so qkv runs at full clock (HAM 8/8); exp fused to
one ACTIVATE per (pair, kt) halving ACT instruction overhead; bias multiplies
split ~50/50 between DVE and GPSIMD; projection fillers redistributed so the
PE never idles >2us (keeps the HAM clock-gate warm).
"""

import numpy as np
import ml_dtypes

B, HH, WW, C = 16, 28, 28, 512
N = HH * WW            # 784 tokens
HEADS, KD = 8, 64
NCORES, BPC = 8, 2     # 8 cores, 2 batches per core
NT, TP = 7, 112        # 784 = 7 tiles of 112 (k / token tiling)
CHUNKS = [(0, 512), (512, 272)]   # q-chunks (PSUM bank = 512 fp32)
CT = 4                 # contraction tiles of 128 over C=512

_CACHE = {}


def _rel_index():
    # Faithful to reference._relative_position_index: token r -> (r%28, r//28)
    t = np.arange(N)
    c0, c1 = t % HH, t // HH
    return ((c0[:, None] - c0[None, :] + HH - 1)
            + (c1[:, None] - c1[None, :] + WW - 1) * (2 * HH - 1))  # [q, k]


def build_nc():
    if 'nc' in _CACHE:
        return _CACHE['nc']
    from contextlib import ExitStack
    import concourse.bacc as bacc
    import concourse.mybir as mybir
    import concourse.tile as tile
    from concourse.alu_op_type import AluOpType

    f32 = mybir.dt.float32
    bf16 = mybir.dt.bfloat16
    EXP = mybir.ActivationFunctionType.Exp

    nc = bacc.Bacc("TRN2", debug=False, enable_asserts=False)
    xT_d = nc.dram_tensor("xT", [BPC, C, N], bf16, kind="ExternalInput").ap()
    wqkv_d = nc.dram_tensor("wqkv", [C, 3 * C], bf16, kind="ExternalInput").ap()
    wout_d = nc.dram_tensor("wout", [C, C], bf16, kind="ExternalInput").ap()
    bias_d = nc.dram_tensor("biasT", [HEADS, N, N], bf16, kind="ExternalInput").ap()
    out_d = nc.dram_tensor("out", [BPC, N, C], bf16, kind="ExternalOutput").ap()

    with tile.TileContext(nc) as tc, ExitStack() as ctx:
        persist = ctx.enter_context(tc.tile_pool(name="persist", bufs=1))
        xT_pool = ctx.enter_context(tc.tile_pool(name="xTp", bufs=8))
        e_pool = ctx.enter_context(tc.tile_pool(name="ep", bufs=1))
        attn_pool = ctx.enter_context(tc.tile_pool(name="atp", bufs=5))
        qkT_pool = ctx.enter_context(tc.tile_pool(name="qkTp", bufs=10))
        den_pool = ctx.enter_context(tc.tile_pool(name="dnp", bufs=1))
        rb_pool = ctx.enter_context(tc.tile_pool(name="rbp", bufs=2))
        osb_pool = ctx.enter_context(tc.tile_pool(name="osbp", bufs=2))
        sc_psum = ctx.enter_context(tc.tile_pool(name="scp", bufs=1, space="PSUM"))
        o_psum = ctx.enter_context(tc.tile_pool(name="opp", bufs=1, space="PSUM"))
        pj_psum = ctx.enter_context(tc.tile_pool(name="pjp", bufs=2, space="PSUM"))

        # ---- weights resident in SBUF ----
        wqkv_sb, wout_sb = [], []
        for ci in range(CT):
            w = persist.tile([128, 3 * C], bf16, tag=f"wqkv{ci}")
            nc.sync.dma_start(w, wqkv_d[ci * 128:(ci + 1) * 128, :])
            wqkv_sb.append(w)
        def load_wout():
            for ci in range(CT):
                w = persist.tile([128, C], bf16, tag=f"wout{ci}",
                                 name=f"wout{ci}")
                nc.sync.dma_start(w, wout_d[ci * 128:(ci + 1) * 128, :])
                wout_sb.append(w)

        # ---- bias tables: one fat DMA per head, resident for both batches --
        bias_sb = {}

        def load_bias(h, eng=None):
            bt = persist.tile([TP, NT * N], bf16, tag=f"bias{h}",
                              name=f"bias{h}")
            (eng or nc.sync).dma_start(
                bt.rearrange("p (kt q) -> p kt q", kt=NT),
                bias_d[h].rearrange("(kt p) q -> p kt q", p=TP))
            bias_sb[h] = bt

        # warm up the ACT exp table load early (overlaps with qkv phase)
        warm = persist.tile([1, 16], f32, tag="warm")
        nc.vector.memset(warm, 0.0)
        nc.scalar.activation(warm, warm, EXP)

        # PE warm-up: dummy matmuls during the initial DMA wait keep the HAM
        # activity window busy so qkv starts at the full 2.4 GHz clock
        dmw = persist.tile([128, 256], bf16, tag="dmw")
        nc.vector.memset(dmw, 0.25)

        def emit_dummies(n, tag):
            for i in range(n):
                dp = pj_psum.tile([128, 512], f32, tag="pj",
                                  name=f"dmy{tag}_{i}")
                nc.tensor.matmul(dp[0:64, 0:256], dmw[:, 0:64], dmw,
                                 start=True, stop=True)

        qkT, vsb, attnT, attn_sb, den_bf = {}, {}, {}, {}, {}
        for b in range(BPC):
            for fi in range(CT):
                attnT[b, fi] = persist.tile(
                    [128, N], bf16, tag=f"attnT{b}_{fi}", name=f"attnT{b}_{fi}")
            den_bf[b, 0] = persist.tile([4, N], bf16, tag=f"den{b}_0",
                                        name=f"den{b}_0")
            for pp in (2, 3):
                den_bf[b, 10 + pp] = persist.tile(
                    [2, N], bf16, tag=f"denp{b}_{pp}", name=f"denp{b}_{pp}")

        xts = {}

        def emit_xt(b):
            tiles = []
            for ci in range(CT):
                xt = xT_pool.tile([128, N], bf16, tag="xT", name=f"xT{b}_{ci}")
                nc.sync.dma_start(xt, xT_d[b, ci * 128:(ci + 1) * 128, :])
                tiles.append(xt)
            xts[b] = tiles

        def emit_qk_tile(b, ft):
            dst = qkT_pool.tile([128, N], bf16, tag="qkT",
                                name=f"qkT{b}_{ft}")
            qkT[b, ft] = dst
            for (c0w, cw) in CHUNKS:
                ps = pj_psum.tile([128, 512], f32, tag="pj",
                                  name=f"pj{b}_{ft}_{c0w}")
                for ci in range(CT):
                    nc.tensor.matmul(
                        ps[:, 0:cw], wqkv_sb[ci][:, ft * 128:(ft + 1) * 128],
                        xts[b][ci][:, c0w:c0w + cw],
                        start=(ci == 0), stop=(ci == CT - 1))
                nc.vector.tensor_copy(dst[:, c0w:c0w + cw], ps[:, 0:cw])

        def emit_v_unit(b, t):
            vt = persist.tile([TP, HEADS, KD + 2], bf16, tag=f"v{b}_{t}",
                              name=f"v{b}_{t}")
            vsb[b, t] = vt
            ps = pj_psum.tile([128, 512], f32, tag="pj", name=f"pv{b}_{t}")
            for ci in range(CT):
                nc.tensor.matmul(
                    ps[0:TP, :], xts[b][ci][:, t * TP:(t + 1) * TP],
                    wqkv_sb[ci][:, 2 * C:3 * C],
                    start=(ci == 0), stop=(ci == CT - 1))
            nc.vector.tensor_copy(
                vt[:, :, 0:KD], ps[0:TP, :].rearrange("p (h d) -> p h d", h=HEADS))
            nc.vector.memset(vt[:, :, KD:KD + 2], 1.0)

        def emit_out_unit(b, t):
            ps = pj_psum.tile([128, 512], f32, tag="pj", name=f"po{b}_{t}")
            for fi in range(CT):
                nc.tensor.matmul(
                    ps[0:TP, :], attnT[b, fi][:, t * TP:(t + 1) * TP],
                    wout_sb[fi], start=(fi == 0), stop=(fi == CT - 1))
            osb = osb_pool.tile([TP, C], bf16, tag="osb")
            nc.vector.tensor_copy(osb, ps[0:TP, :])
            nc.sync.dma_start(out_d[b, t * TP:(t + 1) * TP, :], osb)

        def attention(b, pair, fillers=()):
            fillers = list(fillers)
            h0, h1 = 2 * pair, 2 * pair + 1
            streams = ((0, h0), (1, h1))
            with nc.named_scope(f"attn_b{b}_p{pair}"):
                kT_t, qT_t = qkT[b, 4 + pair], qkT[b, pair]
                ops0, esbs, att = {}, {}, {}
                for kt in range(NT):
                    # scores for BOTH heads into one 4-bank psum tile:
                    # h0 -> cols 0:784, h1 -> cols 1024:1808.  Even/odd head
                    # matmuls adjacent -> disjoint PE row groups (base
                    # partition 0 vs 64) run concurrently.
                    sc = sc_psum.tile([TP, 2048], f32, tag="sc",
                                      name=f"sc{b}_{pair}_{kt}")
                    for (c0w, cw) in CHUNKS:
                        for hs, h in streams:
                            r0 = (h % 2) * 64
                            nc.tensor.matmul(
                                sc[:, 1024 * hs + c0w:1024 * hs + c0w + cw],
                                kT_t[r0:r0 + 64, kt * TP:(kt + 1) * TP],
                                qT_t[r0:r0 + 64, c0w:c0w + cw],
                                start=True, stop=True)
                    # ONE exp over both heads' rows via a strided AP, written
                    # straight into the esb tile; bias multiply runs in-place
                    # (one head on DVE, one on GPSIMD)
                    esb = e_pool.tile([TP, 2 * N], bf16, tag=f"e{kt}",
                                      name=f"e{b}_{pair}_{kt}")
                    esbs[kt] = esb
                    nc.scalar.activation(
                        esb.rearrange("p (c q) -> p c q", c=2),
                        sc.rearrange("p (c q) -> p c q", c=2)[:, :, 0:N], EXP)
                    for hs, h in streams:
                        eng = nc.gpsimd if (kt + hs) % 2 == 0 else nc.vector
                        eng.tensor_tensor(
                            esb[:, hs * N:hs * N + N],
                            esb[:, hs * N:hs * N + N],
                            bias_sb[h][:, kt * N:(kt + 1) * N],
                            AluOpType.mult)
                    # chunk-0 v-matmul trails by two kt so its bias-multiply
                    # has two iterations of slack (covers the slower GPSIMD
                    # tensor_tensor on the offloaded head)
                    if kt >= 2:
                        for hs, h in streams:
                            if kt == 2:
                                ops0[hs] = o_psum.tile([KD + 1, 512], f32,
                                                       tag=f"op{hs}",
                                                       name=f"op0_{b}_{h}")
                            nc.tensor.matmul(
                                ops0[hs], vsb[b, kt - 2][:, h, 0:KD + 1],
                                esbs[kt - 2][:, hs * N:hs * N + 512],
                                start=(kt == 2), stop=False)
                    if fillers:
                        fillers.pop(0)()
                for ktv in (NT - 2, NT - 1):
                    for hs, h in streams:
                        nc.tensor.matmul(
                            ops0[hs], vsb[b, ktv][:, h, 0:KD + 1],
                            esbs[ktv][:, hs * N:hs * N + 512],
                            start=False, stop=(ktv == NT - 1))
                for hs, h in streams:
                    a = attn_pool.tile([KD + 1, N], bf16, tag="attn",
                                       name=f"attn{b}_{h}")
                    att[hs] = a
                    attn_sb[b, h] = a
                    nc.vector.tensor_copy(a[:, 0:512], ops0[hs])
                for hs, h in streams:
                    # chunk-1 accumulator from the pj pool: decouples these
                    # vMMs from the chunk-0 copy's o_psum slot release, and
                    # frees op{hs} for the next pair one hop earlier
                    ops1 = pj_psum.tile([KD + 1, 512], f32, tag="pj",
                                        name=f"op1_{b}_{h}")
                    for kt in range(NT):
                        nc.tensor.matmul(
                            ops1[:, 0:272], vsb[b, kt][:, h, 0:KD + 1],
                            esbs[kt][:, hs * N + 512:hs * N + N],
                            start=(kt == 0), stop=(kt == NT - 1))
                    nc.vector.tensor_copy(att[hs][:, 512:784], ops1[:, 0:272])
                for hs, h in streams:
                    if pair < 2:
                        dden = den_bf[b, 0][(h % 4):(h % 4) + 1, :]
                    else:
                        dden = den_bf[b, 10 + pair][hs:hs + 1, :]
                    nc.sync.dma_start(dden, att[hs][KD:KD + 1, :])
                while fillers:
                    fillers.pop(0)()

        def norm_half(b, half):
            # normalize head pairs (2*half, 2*half+1) of batch b
            from concourse.alu_op_type import AluOpType
            with nc.named_scope(f"norm_b{b}_{half}"):
                dc = den_pool.tile([4, N], f32, tag="dc")
                nc.vector.tensor_copy(dc, den_bf[b, half])
                dr = den_pool.tile([4, N], f32, tag="dr")
                nc.vector.reciprocal_approx_fast(dr, dc)
                db = den_pool.tile([4, N], bf16, tag="db")
                nc.vector.tensor_copy(db, dr)
                for pp in range(2):
                    pair = 2 * half + pp
                    for hs in range(2):
                        h = 2 * pair + hs
                        r0 = hs * 64
                        rb = rb_pool.tile([KD, N], bf16, tag="rb")
                        nc.sync.dma_start(
                            rb, db[2 * pp + hs:2 * pp + hs + 1, None, :]
                            .broadcast_to([1, KD, N]))
                        nc.vector.tensor_tensor(
                            attnT[b, pair][r0:r0 + 64, :],
                            attn_sb[b, h][0:KD, :], rb,
                            AluOpType.mult)

        def norm_pair(b, pair):
            from concourse.alu_op_type import AluOpType
            with nc.named_scope(f"normp_b{b}_{pair}"):
                dc4 = den_pool.tile([4, N], f32, tag="dc", name="dc4")
                dc = dc4[0:2, :]
                nc.vector.tensor_copy(dc, den_bf[b, 10 + pair])
                dr4 = den_pool.tile([4, N], f32, tag="dr", name="dr4")
                dr = dr4[0:2, :]
                nc.vector.reciprocal_approx_fast(dr, dc)
                db4 = den_pool.tile([4, N], bf16, tag="db", name="db4")
                db = db4[0:2, :]
                nc.vector.tensor_copy(db, dr)
                for hs in range(2):
                    h = 2 * pair + hs
                    r0 = hs * 64
                    rb = rb_pool.tile([KD, N], bf16, tag="rb")
                    nc.sync.dma_start(
                        rb, db[hs:hs + 1, None, :].broadcast_to([1, KD, N]))
                    nc.vector.tensor_tensor(
                        attnT[b, pair][r0:r0 + 64, :],
                        attn_sb[b, h][0:KD, :], rb,
                        AluOpType.mult)

        # ---- schedule (b-major; fillers keep the PE warm during attention) --
        emit_xt(0)
        load_bias(0, eng=nc.scalar)
        load_bias(1, eng=nc.scalar)
        emit_dummies(14, "w")          # PE warm-up during the initial DMAs
        with nc.named_scope("qkv_early_b0"):
            emit_qk_tile(0, 0)
            emit_qk_tile(0, 4)
            emit_v_unit(0, 0)
            emit_v_unit(0, 1)
        load_bias(2)
        load_bias(3)
        attention(0, 0, [(lambda t=t: emit_v_unit(0, t)) for t in range(2, 7)]
                  + [lambda: emit_qk_tile(0, 1), lambda: emit_qk_tile(0, 5)])
        load_wout()
        load_bias(4)
        load_bias(5)
        emit_xt(1)
        attention(0, 1, [lambda: emit_qk_tile(0, 2), lambda: emit_qk_tile(0, 6),
                         lambda: emit_v_unit(1, 0), lambda: emit_v_unit(1, 1),
                         lambda: emit_v_unit(1, 2)])
        load_bias(6)
        load_bias(7)
        norm_half(0, 0)
        attention(0, 2, [lambda: emit_qk_tile(0, 3), lambda: emit_qk_tile(0, 7)]
                  + [(lambda t=t: emit_v_unit(1, t)) for t in range(3, 7)])
        norm_pair(0, 2)
        attention(0, 3, [lambda: emit_qk_tile(1, 0), lambda: emit_qk_tile(1, 4),
                         lambda: emit_qk_tile(1, 1), lambda: emit_qk_tile(1, 5)])
        norm_pair(0, 3)
        attention(1, 0, [lambda: emit_out_unit(0, 0), lambda: emit_out_unit(0, 1),
                         lambda: emit_qk_tile(1, 2)])
        attention(1, 1, [lambda: emit_out_unit(0, 2), lambda: emit_out_unit(0, 3),
                         lambda: emit_qk_tile(1, 6)])
        norm_half(1, 0)
        attention(1, 2, [lambda: emit_out_unit(0, 4), lambda: emit_qk_tile(1, 3),
                         lambda: emit_qk_tile(1, 7)])
        norm_pair(1, 2)
        attention(1, 3, [lambda: emit_out_unit(0, 5), lambda: emit_out_unit(0, 6),
                         lambda: emit_dummies(2, "t0")])
        norm_pair(1, 3)
        emit_dummies(4, "t1")
        with nc.named_scope("proj_b1"):
            for t in range(NT):
                emit_out_unit(1, t)
                emit_dummies(1, f"t2_{t}")

    nc.compile()
    _CACHE['nc'] = nc
    return nc


def host_prep(x, w_qkv, pos_table, w_out):
    x = np.asarray(x, np.float32).reshape(B, N, C)
    wq = np.array(np.asarray(w_qkv, np.float32), copy=True)
    wq[:, :C] *= np.float32(1.0 / np.sqrt(KD))
    wq_bf = wq.astype(ml_dtypes.bfloat16)
    idx = _rel_index()
    biasT = np.ascontiguousarray(np.exp(
        np.asarray(pos_table, np.float32)[:, idx].transpose(0, 2, 1)
    )).astype(ml_dtypes.bfloat16)
    wout = np.ascontiguousarray(np.asarray(w_out, np.float32)).astype(
        ml_dtypes.bfloat16)
    in_maps = []
    for c in range(NCORES):
        xT = np.ascontiguousarray(
            x[c * BPC:(c + 1) * BPC].transpose(0, 2, 1)).astype(
                ml_dtypes.bfloat16)  # [2, 512, 784]
        in_maps.append({"xT": xT, "wqkv": wq_bf, "wout": wout, "biasT": biasT})
    return in_maps


def run(in_maps, trace=False, trace_cores=None):
    import concourse.bass_utils as bass_utils
    nc = build_nc()
    return bass_utils.run_bass_kernel_spmd(
        nc, in_maps, core_ids=list(range(NCORES)),
        trace=trace, trace_cores=trace_cores)


def kernel(x, w_qkv, pos_table, w_out):
    in_maps = host_prep(x, w_qkv, pos_table, w_out)
    res = run(in_maps)
    out = np.stack([np.asarray(r["out"], np.float32) for r in res.results])
    return np.ascontiguousarray(out.reshape(B, HH, WW, C))


# revision 18
# speedup vs baseline: 1.1147x; 1.1147x over previous
"""Trainium2 Bass kernel: MHSA with multi-head relative position embedding.

Sharding: data-parallel over batch — 16 batches / 8 cores = 2 batches per core,
each core computes all 8 heads for its 2 batches. No collectives needed.

Math per batch (N=784 tokens, C=512, 8 heads x 64 dim):
  qkv = x @ w_qkv                  (q-columns pre-scaled by 1/8 on host)
  scores_T[k,q] = k_h^T q_h        (head pair packed in one 4-bank PSUM tile;
                                    even/odd head matmuls issued adjacently so
                                    they pack into disjoint PE row groups)
  E = exp(scores_T) * expbias_T    (ONE fused exp on ACT covering both heads'
                                    784-wide rows via a strided AP; bias
                                    multiply split DVE/GPSIMD, bias exp'd host)
  O_T[d,q] = sum_k v_aug[k,d] E[k,q]  with v_aug = [v | 1] -> row 64 = sumexp
  attnT = O_T[0:64] / O_T[64]      (denominator rows DMA-gathered, recip'd,
                                    replicated to 128 partitions with a
                                    stride-0 DMA, DVE mult)
  out = attnT^T stacked over heads @ w_out   (written bf16, host casts f32)

Perf structure vs v1: bias tables loaded ONCE (8 fat DMAs, resident in SBUF
for both batches) instead of per-batch per-tile; PE warmed with dummy matmuls
during the initial DMA wait so qkv runs at full clock (HAM 8/8); exp fused to
one ACTIVATE per (pair, kt) halving ACT instruction overhead; bias multiplies
split ~50/50 between DVE and GPSIMD; projection fillers redistributed so the
PE never idles >2us (keeps the HAM clock-gate warm).
"""

import numpy as np
import ml_dtypes

B, HH, WW, C = 16, 28, 28, 512
N = HH * WW            # 784 tokens
HEADS, KD = 8, 64
NCORES, BPC = 8, 2     # 8 cores, 2 batches per core
NT, TP = 7, 112        # 784 = 7 tiles of 112 (k / token tiling)
CHUNKS = [(0, 512), (512, 272)]   # q-chunks (PSUM bank = 512 fp32)
CT = 4                 # contraction tiles of 128 over C=512

_CACHE = {}


def _rel_index():
    # Faithful to reference._relative_position_index: token r -> (r%28, r//28)
    t = np.arange(N)
    c0, c1 = t % HH, t // HH
    return ((c0[:, None] - c0[None, :] + HH - 1)
            + (c1[:, None] - c1[None, :] + WW - 1) * (2 * HH - 1))  # [q, k]


def build_nc():
    if 'nc' in _CACHE:
        return _CACHE['nc']
    from contextlib import ExitStack
    import concourse.bacc as bacc
    import concourse.mybir as mybir
    import concourse.tile as tile
    from concourse.alu_op_type import AluOpType

    f32 = mybir.dt.float32
    bf16 = mybir.dt.bfloat16
    EXP = mybir.ActivationFunctionType.Exp

    nc = bacc.Bacc("TRN2", debug=False, enable_asserts=False)
    xT_d = nc.dram_tensor("xT", [BPC, C, N], bf16, kind="ExternalInput").ap()
    wqkv_d = nc.dram_tensor("wqkv", [C, 3 * C], bf16, kind="ExternalInput").ap()
    wout_d = nc.dram_tensor("wout", [C, C], bf16, kind="ExternalInput").ap()
    bias_d = nc.dram_tensor("biasT", [HEADS, N, N], bf16, kind="ExternalInput").ap()
    out_d = nc.dram_tensor("out", [BPC, N, C], bf16, kind="ExternalOutput").ap()

    with tile.TileContext(nc) as tc, ExitStack() as ctx:
        persist = ctx.enter_context(tc.tile_pool(name="persist", bufs=1))
        xT_pool = ctx.enter_context(tc.tile_pool(name="xTp", bufs=8))
        e_pool = ctx.enter_context(tc.tile_pool(name="ep", bufs=1))
        attn_pool = ctx.enter_context(tc.tile_pool(name="atp", bufs=5))
        qkT_pool = ctx.enter_context(tc.tile_pool(name="qkTp", bufs=10))
        den_pool = ctx.enter_context(tc.tile_pool(name="dnp", bufs=1))
        rb_pool = ctx.enter_context(tc.tile_pool(name="rbp", bufs=2))
        osb_pool = ctx.enter_context(tc.tile_pool(name="osbp", bufs=2))
        sc_psum = ctx.enter_context(tc.tile_pool(name="scp", bufs=1, space="PSUM"))
        o_psum = ctx.enter_context(tc.tile_pool(name="opp", bufs=1, space="PSUM"))
        pj_psum = ctx.enter_context(tc.tile_pool(name="pjp", bufs=2, space="PSUM"))

        # ---- weights resident in SBUF ----
        wqkv_sb, wout_sb = [], []
        for ci in range(CT):
            w = persist.tile([128, 3 * C], bf16, tag=f"wqkv{ci}")
            nc.sync.dma_start(w, wqkv_d[ci * 128:(ci + 1) * 128, :])
            wqkv_sb.append(w)
        def load_wout():
            for ci in range(CT):
                w = persist.tile([128, C], bf16, tag=f"wout{ci}",
                                 name=f"wout{ci}")
                nc.sync.dma_start(w, wout_d[ci * 128:(ci + 1) * 128, :])
                wout_sb.append(w)

        # ---- bias tables: one fat DMA per head, resident for both batches --
        bias_sb = {}

        def load_bias(h, eng=None):
            bt = persist.tile([TP, NT * N], bf16, tag=f"bias{h}",
                              name=f"bias{h}")
            (eng or nc.sync).dma_start(
                bt.rearrange("p (kt q) -> p kt q", kt=NT),
                bias_d[h].rearrange("(kt p) q -> p kt q", p=TP))
            bias_sb[h] = bt

        # warm up the ACT exp table load early (overlaps with qkv phase)
        warm = persist.tile([1, 16], f32, tag="warm")
        nc.vector.memset(warm, 0.0)
        nc.scalar.activation(warm, warm, EXP)

        # PE warm-up: dummy matmuls during the initial DMA wait keep the HAM
        # activity window busy so qkv starts at the full 2.4 GHz clock
        dmw = persist.tile([128, 256], bf16, tag="dmw")
        nc.vector.memset(dmw, 0.25)
        _dn = [0]

        def emit_dummies(n, tag="d"):
            for i in range(n):
                _dn[0] += 1
                dp = pj_psum.tile([128, 512], f32, tag="pj",
                                  name=f"dmy{tag}_{_dn[0]}")
                nc.tensor.matmul(dp[0:64, 0:256], dmw[:, 0:64], dmw,
                                 start=True, stop=True)

        qkT, vsb, attnT, attn_sb, den_bf = {}, {}, {}, {}, {}
        for b in range(BPC):
            for fi in range(CT):
                attnT[b, fi] = persist.tile(
                    [128, N], bf16, tag=f"attnT{b}_{fi}", name=f"attnT{b}_{fi}")
            den_bf[b, 0] = persist.tile([4, N], bf16, tag=f"den{b}_0",
                                        name=f"den{b}_0")
            for pp in (2, 3):
                den_bf[b, 10 + pp] = persist.tile(
                    [2, N], bf16, tag=f"denp{b}_{pp}", name=f"denp{b}_{pp}")

        xts = {}

        def emit_xt(b):
            tiles = []
            for ci in range(CT):
                xt = xT_pool.tile([128, N], bf16, tag="xT", name=f"xT{b}_{ci}")
                nc.sync.dma_start(xt, xT_d[b, ci * 128:(ci + 1) * 128, :])
                tiles.append(xt)
            xts[b] = tiles

        def emit_qk_chunk(b, ft, ic):
            # half a qk feature tile (one q-chunk) -> finer filler granularity
            if (b, ft) not in qkT:
                qkT[b, ft] = qkT_pool.tile([128, N], bf16, tag="qkT",
                                           name=f"qkT{b}_{ft}")
            dst = qkT[b, ft]
            c0w, cw = CHUNKS[ic]
            ps = pj_psum.tile([128, 512], f32, tag="pj",
                              name=f"pj{b}_{ft}_{c0w}")
            for ci in range(CT):
                nc.tensor.matmul(
                    ps[:, 0:cw], wqkv_sb[ci][:, ft * 128:(ft + 1) * 128],
                    xts[b][ci][:, c0w:c0w + cw],
                    start=(ci == 0), stop=(ci == CT - 1))
            nc.vector.tensor_copy(dst[:, c0w:c0w + cw], ps[:, 0:cw])

        def emit_qk_tile(b, ft):
            emit_qk_chunk(b, ft, 0)
            emit_qk_chunk(b, ft, 1)

        def emit_v_unit(b, t):
            vt = persist.tile([TP, HEADS, KD + 2], bf16, tag=f"v{b}_{t}",
                              name=f"v{b}_{t}")
            vsb[b, t] = vt
            ps = pj_psum.tile([128, 512], f32, tag="pj", name=f"pv{b}_{t}")
            for ci in range(CT):
                nc.tensor.matmul(
                    ps[0:TP, :], xts[b][ci][:, t * TP:(t + 1) * TP],
                    wqkv_sb[ci][:, 2 * C:3 * C],
                    start=(ci == 0), stop=(ci == CT - 1))
            nc.vector.tensor_copy(
                vt[:, :, 0:KD], ps[0:TP, :].rearrange("p (h d) -> p h d", h=HEADS))
            nc.vector.memset(vt[:, :, KD:KD + 2], 1.0)

        def emit_out_unit(b, t):
            ps = pj_psum.tile([128, 512], f32, tag="pj", name=f"po{b}_{t}")
            for fi in range(CT):
                nc.tensor.matmul(
                    ps[0:TP, :], attnT[b, fi][:, t * TP:(t + 1) * TP],
                    wout_sb[fi], start=(fi == 0), stop=(fi == CT - 1))
            osb = osb_pool.tile([TP, C], bf16, tag="osb")
            nc.vector.tensor_copy(osb, ps[0:TP, :])
            nc.sync.dma_start(out_d[b, t * TP:(t + 1) * TP, :], osb)

        def attention(b, pair, fillers=()):
            fillers = list(fillers)
            h0, h1 = 2 * pair, 2 * pair + 1
            streams = ((0, h0), (1, h1))
            with nc.named_scope(f"attn_b{b}_p{pair}"):
                kT_t, qT_t = qkT[b, 4 + pair], qkT[b, pair]
                ops0, esbs, att, scs = {}, {}, {}, {}

                def sc_alloc(hs, kt):
                    t = sc_psum.tile([TP, 1024], f32, tag=f"sc{hs}",
                                     name=f"sc{b}_{pair}_{hs}_{kt}")
                    scs[hs, kt] = t
                    return t

                # Software-pipelined over kt: ACT(h0,kt) runs while the PE
                # writes scores for (h1,kt) and (h0,kt+1) -- issued adjacently
                # so the two heads' matmuls pack into disjoint PE row groups
                # (base partition 0 vs 64) and run concurrently.  ACT never
                # waits on scores; scores never wait on ACT (2 psum tiles).
                sc_alloc(0, 0)
                for (c0w, cw) in CHUNKS:
                    nc.tensor.matmul(
                        scs[0, 0][:, c0w:c0w + cw],
                        kT_t[0:64, 0:TP], qT_t[0:64, c0w:c0w + cw],
                        start=True, stop=True)
                for kt in range(NT):
                    # exp of h0's scores; bias-mult immediately behind it
                    esb0 = e_pool.tile([TP, N], bf16, tag=f"e0_{kt}",
                                       name=f"e{b}_{pair}_0_{kt}")
                    esbs[0, kt] = esb0
                    nc.scalar.activation(esb0, scs[0, kt][:, 0:N], EXP)
                    eng = nc.gpsimd if kt % 2 == 0 else nc.vector
                    eng.tensor_tensor(esb0, esb0,
                                      bias_sb[h0][:, kt * N:(kt + 1) * N],
                                      AluOpType.mult)
                    # h1's scores stream while ACT chews on h0
                    sc_alloc(1, kt)
                    for (c0w, cw) in CHUNKS:
                        nc.tensor.matmul(
                            scs[1, kt][:, c0w:c0w + cw],
                            kT_t[64:128, kt * TP:(kt + 1) * TP],
                            qT_t[64:128, c0w:c0w + cw],
                            start=True, stop=True)
                    esb1 = e_pool.tile([TP, N], bf16, tag=f"e1_{kt}",
                                       name=f"e{b}_{pair}_1_{kt}")
                    esbs[1, kt] = esb1
                    nc.scalar.activation(esb1, scs[1, kt][:, 0:N], EXP)
                    eng = nc.gpsimd if kt % 2 == 1 else nc.vector
                    eng.tensor_tensor(esb1, esb1,
                                      bias_sb[h1][:, kt * N:(kt + 1) * N],
                                      AluOpType.mult)
                    # chunk-0 v-matmul trails by two kt so its bias-multiply
                    # has two iterations of slack (covers the slower GPSIMD
                    # tensor_tensor on the offloaded head)
                    if kt >= 2:
                        for hs, h in streams:
                            if kt == 2:
                                ops0[hs] = o_psum.tile([KD + 1, 512], f32,
                                                       tag=f"op{hs}",
                                                       name=f"op0_{b}_{h}")
                            nc.tensor.matmul(
                                ops0[hs], vsb[b, kt - 2][:, h, 0:KD + 1],
                                esbs[hs, kt - 2][:, 0:512],
                                start=(kt == 2), stop=False)
                    # h0's scores for kt+1 run while ACT chews on h1
                    if kt + 1 < NT:
                        sc_alloc(0, kt + 1)
                        for (c0w, cw) in CHUNKS:
                            nc.tensor.matmul(
                                scs[0, kt + 1][:, c0w:c0w + cw],
                                kT_t[0:64, (kt + 1) * TP:(kt + 2) * TP],
                                qT_t[0:64, c0w:c0w + cw],
                                start=True, stop=True)
                    if fillers:
                        fillers.pop(0)()
                for ktv in (NT - 2, NT - 1):
                    for hs, h in streams:
                        nc.tensor.matmul(
                            ops0[hs], vsb[b, ktv][:, h, 0:KD + 1],
                            esbs[hs, ktv][:, 0:512],
                            start=False, stop=(ktv == NT - 1))
                for hs, h in streams:
                    a = attn_pool.tile([KD + 1, N], bf16, tag="attn",
                                       name=f"attn{b}_{h}")
                    att[hs] = a
                    attn_sb[b, h] = a
                    nc.vector.tensor_copy(a[:, 0:512], ops0[hs])
                for hs, h in streams:
                    # chunk-1 accumulator from the pj pool: decouples these
                    # vMMs from the chunk-0 copy's o_psum slot release, and
                    # frees op{hs} for the next pair one hop earlier
                    ops1 = pj_psum.tile([KD + 1, 512], f32, tag="pj",
                                        name=f"op1_{b}_{h}")
                    for kt in range(NT):
                        nc.tensor.matmul(
                            ops1[:, 0:272], vsb[b, kt][:, h, 0:KD + 1],
                            esbs[hs, kt][:, 512:784],
                            start=(kt == 0), stop=(kt == NT - 1))
                    nc.vector.tensor_copy(att[hs][:, 512:784], ops1[:, 0:272])
                for hs, h in streams:
                    if pair < 2:
                        dden = den_bf[b, 0][(h % 4):(h % 4) + 1, :]
                    else:
                        dden = den_bf[b, 10 + pair][hs:hs + 1, :]
                    nc.sync.dma_start(dden, att[hs][KD:KD + 1, :])
                while fillers:
                    fillers.pop(0)()

        def norm_half(b, half):
            # normalize head pairs (2*half, 2*half+1) of batch b
            from concourse.alu_op_type import AluOpType
            with nc.named_scope(f"norm_b{b}_{half}"):
                dc = den_pool.tile([4, N], f32, tag="dc")
                nc.vector.tensor_copy(dc, den_bf[b, half])
                dr = den_pool.tile([4, N], f32, tag="dr")
                nc.vector.reciprocal_approx_fast(dr, dc)
                db = den_pool.tile([4, N], bf16, tag="db")
                nc.vector.tensor_copy(db, dr)
                for pp in range(2):
                    pair = 2 * half + pp
                    for hs in range(2):
                        h = 2 * pair + hs
                        r0 = hs * 64
                        rb = rb_pool.tile([KD, N], bf16, tag="rb")
                        nc.sync.dma_start(
                            rb, db[2 * pp + hs:2 * pp + hs + 1, None, :]
                            .broadcast_to([1, KD, N]))
                        nc.vector.tensor_tensor(
                            attnT[b, pair][r0:r0 + 64, :],
                            attn_sb[b, h][0:KD, :], rb,
                            AluOpType.mult)

        def norm_pair(b, pair):
            from concourse.alu_op_type import AluOpType
            with nc.named_scope(f"normp_b{b}_{pair}"):
                dc4 = den_pool.tile([4, N], f32, tag="dc", name="dc4")
                dc = dc4[0:2, :]
                nc.vector.tensor_copy(dc, den_bf[b, 10 + pair])
                dr4 = den_pool.tile([4, N], f32, tag="dr", name="dr4")
                dr = dr4[0:2, :]
                nc.vector.reciprocal_approx_fast(dr, dc)
                db4 = den_pool.tile([4, N], bf16, tag="db", name="db4")
                db = db4[0:2, :]
                nc.vector.tensor_copy(db, dr)
                for hs in range(2):
                    h = 2 * pair + hs
                    r0 = hs * 64
                    rb = rb_pool.tile([KD, N], bf16, tag="rb")
                    nc.sync.dma_start(
                        rb, db[hs:hs + 1, None, :].broadcast_to([1, KD, N]))
                    nc.vector.tensor_tensor(
                        attnT[b, pair][r0:r0 + 64, :],
                        attn_sb[b, h][0:KD, :], rb,
                        AluOpType.mult)

        # ---- schedule (b-major; fillers keep the PE warm during attention) --
        def qkh(b, ft, ic):
            return lambda: emit_qk_chunk(b, ft, ic)

        def vu(b, t):
            return lambda: emit_v_unit(b, t)

        def ou(b, t):
            return lambda: emit_out_unit(b, t)

        def dmy():
            return lambda: emit_dummies(1)

        emit_xt(0)
        load_bias(0, eng=nc.scalar)
        load_bias(1, eng=nc.scalar)
        emit_dummies(24, "w")          # PE warm-up during the initial DMAs
        with nc.named_scope("qkv_early_b0"):
            emit_qk_tile(0, 0)
            emit_qk_tile(0, 4)
            emit_v_unit(0, 0)
            emit_v_unit(0, 1)
        load_bias(2)
        load_bias(3)
        attention(0, 0, [vu(0, 2), vu(0, 3), vu(0, 4), vu(0, 5), vu(0, 6),
                         qkh(0, 1, 0), qkh(0, 1, 1),
                         qkh(0, 5, 0), qkh(0, 5, 1)])
        load_wout()
        load_bias(4)
        load_bias(5)
        emit_xt(1)
        attention(0, 1, [qkh(0, 2, 0), qkh(0, 2, 1), qkh(0, 6, 0), qkh(0, 6, 1),
                         vu(1, 0), vu(1, 1), vu(1, 2)])
        load_bias(6)
        load_bias(7)
        norm_half(0, 0)
        attention(0, 2, [qkh(0, 3, 0), qkh(0, 3, 1), qkh(0, 7, 0), qkh(0, 7, 1),
                         vu(1, 3), vu(1, 4), vu(1, 5), vu(1, 6)])
        norm_pair(0, 2)
        attention(0, 3, [qkh(1, 0, 0), qkh(1, 0, 1), qkh(1, 4, 0), qkh(1, 4, 1),
                         qkh(1, 1, 0), qkh(1, 1, 1), qkh(1, 5, 0), qkh(1, 5, 1)])
        norm_pair(0, 3)
        attention(1, 0, [ou(0, 0), ou(0, 1), qkh(1, 2, 0), qkh(1, 2, 1),
                         dmy(), dmy()])
        attention(1, 1, [ou(0, 2), ou(0, 3), qkh(1, 6, 0), qkh(1, 6, 1),
                         dmy(), dmy()])
        norm_half(1, 0)
        attention(1, 2, [ou(0, 4), qkh(1, 3, 0), qkh(1, 3, 1),
                         qkh(1, 7, 0), qkh(1, 7, 1), dmy(), dmy()])
        norm_pair(1, 2)
        attention(1, 3, [ou(0, 5), ou(0, 6), dmy(), dmy(), dmy(), dmy()])
        norm_pair(1, 3)
        emit_dummies(2, "t1")
        with nc.named_scope("proj_b1"):
            for t in range(NT):
                emit_out_unit(1, t)
                emit_dummies(1, f"t2_{t}")

    nc.compile()
    _CACHE['nc'] = nc
    return nc


def host_prep(x, w_qkv, pos_table, w_out):
    x = np.asarray(x, np.float32).reshape(B, N, C)
    wq = np.array(np.asarray(w_qkv, np.float32), copy=True)
    wq[:, :C] *= np.float32(1.0 / np.sqrt(KD))
    wq_bf = wq.astype(ml_dtypes.bfloat16)
    idx = _rel_index()
    biasT = np.ascontiguousarray(np.exp(
        np.asarray(pos_table, np.float32)[:, idx].transpose(0, 2, 1)
    )).astype(ml_dtypes.bfloat16)
    wout = np.ascontiguousarray(np.asarray(w_out, np.float32)).astype(
        ml_dtypes.bfloat16)
    in_maps = []
    for c in range(NCORES):
        xT = np.ascontiguousarray(
            x[c * BPC:(c + 1) * BPC].transpose(0, 2, 1)).astype(
                ml_dtypes.bfloat16)  # [2, 512, 784]
        in_maps.append({"xT": xT, "wqkv": wq_bf, "wout": wout, "biasT": biasT})
    return in_maps


def run(in_maps, trace=False, trace_cores=None):
    import concourse.bass_utils as bass_utils
    nc = build_nc()
    return bass_utils.run_bass_kernel_spmd(
        nc, in_maps, core_ids=list(range(NCORES)),
        trace=trace, trace_cores=trace_cores)


def kernel(x, w_qkv, pos_table, w_out):
    in_maps = host_prep(x, w_qkv, pos_table, w_out)
    res = run(in_maps)
    out = np.stack([np.asarray(r["out"], np.float32) for r in res.results])
    return np.ascontiguousarray(out.reshape(B, HH, WW, C))


# revision 26
# speedup vs baseline: 1.1679x; 1.0477x over previous
"""Trainium2 Bass kernel: MHSA with multi-head relative position embedding.

Sharding: data-parallel over batch — 16 batches / 8 cores = 2 batches per core,
each core computes all 8 heads for its 2 batches. No collectives needed.

Math per batch (N=784 tokens, C=512, 8 heads x 64 dim):
  qkv = x @ w_qkv                  (q-columns pre-scaled by 1/8 on host)
  scores_T[k,q] = k_h^T q_h        (head pair packed in one 4-bank PSUM tile;
                                    even/odd head matmuls issued adjacently so
                                    they pack into disjoint PE row groups)
  E = exp(scores_T) * expbias_T    (ONE fused exp on ACT covering both heads'
                                    784-wide rows via a strided AP; bias
                                    multiply split DVE/GPSIMD, bias exp'd host)
  O_T[d,q] = sum_k v_aug[k,d] E[k,q]  with v_aug = [v | 1] -> row 64 = sumexp
  attnT = O_T[0:64] / O_T[64]      (denominator rows DMA-gathered, recip'd,
                                    replicated to 128 partitions with a
                                    stride-0 DMA, DVE mult)
  out = attnT^T stacked over heads @ w_out   (written bf16, host casts f32)

Perf structure vs v1: bias tables loaded ONCE (8 fat DMAs, resident in SBUF
for both batches) instead of per-batch per-tile; PE warmed with dummy matmuls
during the initial DMA wait so qkv runs at full clock (HAM 8/8); exp fused to
one ACTIVATE per (pair, kt) halving ACT instruction overhead; bias multiplies
split ~50/50 between DVE and GPSIMD; projection fillers redistributed so the
PE never idles >2us (keeps the HAM clock-gate warm).
"""

import numpy as np
import ml_dtypes

B, HH, WW, C = 16, 28, 28, 512
N = HH * WW            # 784 tokens
HEADS, KD = 8, 64
NCORES, BPC = 8, 2     # 8 cores, 2 batches per core
NT, TP = 7, 112        # 784 = 7 tiles of 112 (k / token tiling)
CHUNKS = [(0, 512), (512, 272)]   # q-chunks (PSUM bank = 512 fp32)
CT = 4                 # contraction tiles of 128 over C=512

_CACHE = {}


def _rel_index():
    # Faithful to reference._relative_position_index: token r -> (r%28, r//28)
    t = np.arange(N)
    c0, c1 = t % HH, t // HH
    return ((c0[:, None] - c0[None, :] + HH - 1)
            + (c1[:, None] - c1[None, :] + WW - 1) * (2 * HH - 1))  # [q, k]


def build_nc():
    if 'nc' in _CACHE:
        return _CACHE['nc']
    from contextlib import ExitStack
    import concourse.bacc as bacc
    import concourse.mybir as mybir
    import concourse.tile as tile
    from concourse.alu_op_type import AluOpType

    f32 = mybir.dt.float32
    bf16 = mybir.dt.bfloat16
    EXP = mybir.ActivationFunctionType.Exp

    nc = bacc.Bacc("TRN2", debug=False, enable_asserts=False)
    xT_d = nc.dram_tensor("xT", [BPC, C, N], bf16, kind="ExternalInput").ap()
    wqkv_d = nc.dram_tensor("wqkv", [C, 3 * C], bf16, kind="ExternalInput").ap()
    wout_d = nc.dram_tensor("wout", [C, C], bf16, kind="ExternalInput").ap()
    bias_d = nc.dram_tensor("biasT", [HEADS, N, N], bf16, kind="ExternalInput").ap()
    out_d = nc.dram_tensor("out", [BPC, N, C], bf16, kind="ExternalOutput").ap()

    with tile.TileContext(nc) as tc, ExitStack() as ctx:
        persist = ctx.enter_context(tc.tile_pool(name="persist", bufs=1))
        xT_pool = ctx.enter_context(tc.tile_pool(name="xTp", bufs=8))
        e_pool = ctx.enter_context(tc.tile_pool(name="ep", bufs=1))
        attn_pool = ctx.enter_context(tc.tile_pool(name="atp", bufs=5))
        qkT_pool = ctx.enter_context(tc.tile_pool(name="qkTp", bufs=10))
        den_pool = ctx.enter_context(tc.tile_pool(name="dnp", bufs=1))
        rb_pool = ctx.enter_context(tc.tile_pool(name="rbp", bufs=2))
        osb_pool = ctx.enter_context(tc.tile_pool(name="osbp", bufs=2))
        sc_psum = ctx.enter_context(tc.tile_pool(name="scp", bufs=1, space="PSUM"))
        o_psum = ctx.enter_context(tc.tile_pool(name="opp", bufs=1, space="PSUM"))
        pj_psum = ctx.enter_context(tc.tile_pool(name="pjp", bufs=2, space="PSUM"))

        # ---- weights resident in SBUF ----
        # (wqkv DMAs are interleaved with the xT loads in the schedule so the
        # first qkv matmul chain is DMA-paced, not serialized)
        wqkv_sb, wout_sb = [], []

        def load_wqkv(ci):
            w = persist.tile([128, 3 * C], bf16, tag=f"wqkv{ci}",
                             name=f"wqkv{ci}")
            nc.sync.dma_start(w, wqkv_d[ci * 128:(ci + 1) * 128, :])
            wqkv_sb.append(w)
        def load_wout():
            for ci in range(CT):
                w = persist.tile([128, C], bf16, tag=f"wout{ci}",
                                 name=f"wout{ci}")
                nc.sync.dma_start(w, wout_d[ci * 128:(ci + 1) * 128, :])
                wout_sb.append(w)

        # ---- bias tables: one fat DMA per head, resident for both batches --
        bias_sb = {}

        def load_bias(h, eng=None):
            bt = persist.tile([TP, NT * N], bf16, tag=f"bias{h}",
                              name=f"bias{h}")
            (eng or nc.sync).dma_start(
                bt.rearrange("p (kt q) -> p kt q", kt=NT),
                bias_d[h].rearrange("(kt p) q -> p kt q", p=TP))
            bias_sb[h] = bt

        # warm up the ACT exp table load early (overlaps with qkv phase)
        warm = persist.tile([1, 16], f32, tag="warm")
        nc.vector.memset(warm, 0.0)
        nc.scalar.activation(warm, warm, EXP)

        # PE warm-up: dummy matmuls during the initial DMA wait keep the HAM
        # activity window busy so qkv starts at the full 2.4 GHz clock
        dmw = persist.tile([128, 256], bf16, tag="dmw")
        nc.vector.memset(dmw, 0.25)
        _dn = [0]

        def emit_dummies(n, tag="d"):
            for i in range(n):
                _dn[0] += 1
                dp = pj_psum.tile([128, 512], f32, tag="pj",
                                  name=f"dmy{tag}_{_dn[0]}")
                nc.tensor.matmul(dp[0:64, 0:256], dmw[:, 0:64], dmw,
                                 start=True, stop=True)

        qkT, vsb, attnT, attn_sb, den_bf = {}, {}, {}, {}, {}
        for b in range(BPC):
            for fi in range(CT):
                attnT[b, fi] = persist.tile(
                    [128, N], bf16, tag=f"attnT{b}_{fi}", name=f"attnT{b}_{fi}")
            den_bf[b, 0] = persist.tile([4, N], bf16, tag=f"den{b}_0",
                                        name=f"den{b}_0")
            for pp in (2, 3):
                den_bf[b, 10 + pp] = persist.tile(
                    [2, N], bf16, tag=f"denp{b}_{pp}", name=f"denp{b}_{pp}")

        xts = {}

        def emit_xt(b, interleave_wqkv=False):
            tiles = []
            for ci in range(CT):
                if interleave_wqkv:
                    load_wqkv(ci)
                xt = xT_pool.tile([128, N], bf16, tag="xT", name=f"xT{b}_{ci}")
                nc.sync.dma_start(xt, xT_d[b, ci * 128:(ci + 1) * 128, :])
                tiles.append(xt)
            xts[b] = tiles

        def emit_qk_chunk(b, ft, ic):
            # half a qk feature tile (one q-chunk) -> finer filler granularity
            if (b, ft) not in qkT:
                qkT[b, ft] = qkT_pool.tile([128, N], bf16, tag="qkT",
                                           name=f"qkT{b}_{ft}")
            dst = qkT[b, ft]
            c0w, cw = CHUNKS[ic]
            ps = pj_psum.tile([128, 512], f32, tag="pj",
                              name=f"pj{b}_{ft}_{c0w}")
            for ci in range(CT):
                nc.tensor.matmul(
                    ps[:, 0:cw], wqkv_sb[ci][:, ft * 128:(ft + 1) * 128],
                    xts[b][ci][:, c0w:c0w + cw],
                    start=(ci == 0), stop=(ci == CT - 1))
            nc.vector.tensor_copy(dst[:, c0w:c0w + cw], ps[:, 0:cw])

        def emit_qk_tile(b, ft):
            emit_qk_chunk(b, ft, 0)
            emit_qk_chunk(b, ft, 1)

        def emit_v_unit(b, t):
            vt = persist.tile([TP, HEADS, KD + 2], bf16, tag=f"v{b}_{t}",
                              name=f"v{b}_{t}")
            vsb[b, t] = vt
            ps = pj_psum.tile([128, 512], f32, tag="pj", name=f"pv{b}_{t}")
            for ci in range(CT):
                nc.tensor.matmul(
                    ps[0:TP, :], xts[b][ci][:, t * TP:(t + 1) * TP],
                    wqkv_sb[ci][:, 2 * C:3 * C],
                    start=(ci == 0), stop=(ci == CT - 1))
            nc.vector.tensor_copy(
                vt[:, :, 0:KD], ps[0:TP, :].rearrange("p (h d) -> p h d", h=HEADS))
            nc.vector.memset(vt[:, :, KD:KD + 2], 1.0)

        def emit_out_unit(b, t, scalar_copy=False):
            ps = pj_psum.tile([128, 512], f32, tag="pj", name=f"po{b}_{t}")
            for fi in range(CT):
                nc.tensor.matmul(
                    ps[0:TP, :], attnT[b, fi][:, t * TP:(t + 1) * TP],
                    wout_sb[fi], start=(fi == 0), stop=(fi == CT - 1))
            osb = osb_pool.tile([TP, C], bf16, tag="osb")
            if scalar_copy:
                # final projection runs after all exps -- ACT is idle there
                nc.scalar.copy(osb, ps[0:TP, :])
            else:
                nc.vector.tensor_copy(osb, ps[0:TP, :])
            nc.sync.dma_start(out_d[b, t * TP:(t + 1) * TP, :], osb)

        pro_sc = {}

        def attn_prologue(b, pair):
            # first head-0 score tile of a pair; hoisted into the previous
            # pair's tail so the ACT pipeline never drains across pairs
            if (b, pair) in pro_sc:
                return
            t = sc_psum.tile([TP, 1024], f32, tag="sc0",
                             name=f"sc{b}_{pair}_0_0")
            pro_sc[b, pair] = t
            kT_t, qT_t = qkT[b, 4 + pair], qkT[b, pair]
            for (c0w, cw) in CHUNKS:
                nc.tensor.matmul(
                    t[:, c0w:c0w + cw],
                    kT_t[0:64, 0:TP], qT_t[0:64, c0w:c0w + cw],
                    start=True, stop=True)

        def attention(b, pair, fillers=(), nxt=None):
            fillers = list(fillers)
            h0, h1 = 2 * pair, 2 * pair + 1
            streams = ((0, h0), (1, h1))
            with nc.named_scope(f"attn_b{b}_p{pair}"):
                kT_t, qT_t = qkT[b, 4 + pair], qkT[b, pair]
                ops0, esbs, att, scs = {}, {}, {}, {}

                def sc_alloc(hs, kt):
                    t = sc_psum.tile([TP, 1024], f32, tag=f"sc{hs}",
                                     name=f"sc{b}_{pair}_{hs}_{kt}")
                    scs[hs, kt] = t
                    return t

                # Software-pipelined over kt: ACT(h0,kt) runs while the PE
                # writes scores for (h1,kt); ACT(h1,kt) covers (h0,kt+1).
                # ACT never waits on scores; scores never wait on ACT
                # (2 rotating psum tiles).
                attn_prologue(b, pair)
                scs[0, 0] = pro_sc[b, pair]
                for kt in range(NT):
                    # exp of h0's scores; bias-mult immediately behind it
                    esb0 = e_pool.tile([TP, N], bf16, tag=f"e0_{kt}",
                                       name=f"e{b}_{pair}_0_{kt}")
                    esbs[0, kt] = esb0
                    nc.scalar.activation(esb0, scs[0, kt][:, 0:N], EXP)
                    nc.vector.tensor_tensor(
                        esb0, esb0, bias_sb[h0][:, kt * N:(kt + 1) * N],
                        AluOpType.mult)
                    # h1's scores stream while ACT chews on h0
                    sc_alloc(1, kt)
                    for (c0w, cw) in CHUNKS:
                        nc.tensor.matmul(
                            scs[1, kt][:, c0w:c0w + cw],
                            kT_t[64:128, kt * TP:(kt + 1) * TP],
                            qT_t[64:128, c0w:c0w + cw],
                            start=True, stop=True)
                    esb1 = e_pool.tile([TP, N], bf16, tag=f"e1_{kt}",
                                       name=f"e{b}_{pair}_1_{kt}")
                    esbs[1, kt] = esb1
                    nc.scalar.activation(esb1, scs[1, kt][:, 0:N], EXP)
                    nc.gpsimd.tensor_tensor(
                        esb1, esb1, bias_sb[h1][:, kt * N:(kt + 1) * N],
                        AluOpType.mult)
                    # chunk-0 v-matmul trails by two kt so its bias-multiply
                    # has two iterations of slack (covers the slower GPSIMD
                    # tensor_tensor on the offloaded head)
                    if kt >= 2:
                        for hs, h in streams:
                            if kt == 2:
                                ops0[hs] = o_psum.tile([KD + 1, 512], f32,
                                                       tag=f"op{hs}",
                                                       name=f"op0_{b}_{h}")
                            nc.tensor.matmul(
                                ops0[hs], vsb[b, kt - 2][:, h, 0:KD + 1],
                                esbs[hs, kt - 2][:, 0:512],
                                start=(kt == 2), stop=False)
                    # h0's scores for kt+1 run while ACT chews on h1
                    if kt + 1 < NT:
                        sc_alloc(0, kt + 1)
                        for (c0w, cw) in CHUNKS:
                            nc.tensor.matmul(
                                scs[0, kt + 1][:, c0w:c0w + cw],
                                kT_t[0:64, (kt + 1) * TP:(kt + 2) * TP],
                                qT_t[0:64, c0w:c0w + cw],
                                start=True, stop=True)
                    if fillers:
                        fillers.pop(0)()
                for ktv in (NT - 2, NT - 1):
                    for hs, h in streams:
                        nc.tensor.matmul(
                            ops0[hs], vsb[b, ktv][:, h, 0:KD + 1],
                            esbs[hs, ktv][:, 0:512],
                            start=False, stop=(ktv == NT - 1))
                if nxt is not None:
                    attn_prologue(*nxt)
                for hs, h in streams:
                    a = attn_pool.tile([KD + 1, N], bf16, tag="attn",
                                       name=f"attn{b}_{h}")
                    att[hs] = a
                    attn_sb[b, h] = a
                    nc.vector.tensor_copy(a[:, 0:512], ops0[hs])
                for hs, h in streams:
                    # chunk-1 accumulator from the pj pool: decouples these
                    # vMMs from the chunk-0 copy's o_psum slot release, and
                    # frees op{hs} for the next pair one hop earlier
                    ops1 = pj_psum.tile([KD + 1, 512], f32, tag="pj",
                                        name=f"op1_{b}_{h}")
                    for kt in range(NT):
                        nc.tensor.matmul(
                            ops1[:, 0:272], vsb[b, kt][:, h, 0:KD + 1],
                            esbs[hs, kt][:, 512:784],
                            start=(kt == 0), stop=(kt == NT - 1))
                    nc.vector.tensor_copy(att[hs][:, 512:784], ops1[:, 0:272])
                for hs, h in streams:
                    if pair < 2:
                        dden = den_bf[b, 0][(h % 4):(h % 4) + 1, :]
                    else:
                        dden = den_bf[b, 10 + pair][hs:hs + 1, :]
                    nc.sync.dma_start(dden, att[hs][KD:KD + 1, :])
                while fillers:
                    fillers.pop(0)()

        def norm_half(b, half):
            # normalize head pairs (2*half, 2*half+1) of batch b
            from concourse.alu_op_type import AluOpType
            with nc.named_scope(f"norm_b{b}_{half}"):
                dc = den_pool.tile([4, N], f32, tag="dc")
                nc.vector.tensor_copy(dc, den_bf[b, half])
                dr = den_pool.tile([4, N], f32, tag="dr")
                nc.vector.reciprocal_approx_fast(dr, dc)
                db = den_pool.tile([4, N], bf16, tag="db")
                nc.vector.tensor_copy(db, dr)
                for pp in range(2):
                    pair = 2 * half + pp
                    for hs in range(2):
                        h = 2 * pair + hs
                        r0 = hs * 64
                        rb = rb_pool.tile([KD, N], bf16, tag="rb")
                        nc.sync.dma_start(
                            rb, db[2 * pp + hs:2 * pp + hs + 1, None, :]
                            .broadcast_to([1, KD, N]))
                        nc.vector.tensor_tensor(
                            attnT[b, pair][r0:r0 + 64, :],
                            attn_sb[b, h][0:KD, :], rb,
                            AluOpType.mult)

        def norm_pair(b, pair):
            from concourse.alu_op_type import AluOpType
            with nc.named_scope(f"normp_b{b}_{pair}"):
                dc4 = den_pool.tile([4, N], f32, tag="dc", name="dc4")
                dc = dc4[0:2, :]
                nc.vector.tensor_copy(dc, den_bf[b, 10 + pair])
                dr4 = den_pool.tile([4, N], f32, tag="dr", name="dr4")
                dr = dr4[0:2, :]
                nc.vector.reciprocal_approx_fast(dr, dc)
                db4 = den_pool.tile([4, N], bf16, tag="db", name="db4")
                db = db4[0:2, :]
                nc.vector.tensor_copy(db, dr)
                for hs in range(2):
                    h = 2 * pair + hs
                    r0 = hs * 64
                    rb = rb_pool.tile([KD, N], bf16, tag="rb")
                    nc.sync.dma_start(
                        rb, db[hs:hs + 1, None, :].broadcast_to([1, KD, N]))
                    nc.vector.tensor_tensor(
                        attnT[b, pair][r0:r0 + 64, :],
                        attn_sb[b, h][0:KD, :], rb,
                        AluOpType.mult)

        # ---- schedule (b-major; fillers keep the PE warm during attention) --
        def qkh(b, ft, ic):
            return lambda: emit_qk_chunk(b, ft, ic)

        def vu(b, t):
            return lambda: emit_v_unit(b, t)

        def ou(b, t):
            return lambda: emit_out_unit(b, t)

        def dmy():
            return lambda: emit_dummies(1)

        emit_xt(0, interleave_wqkv=True)
        load_bias(0)
        load_bias(1)
        emit_dummies(24, "w")          # PE warm-up during the initial DMAs
        with nc.named_scope("qkv_early_b0"):
            emit_qk_tile(0, 0)
            emit_qk_tile(0, 4)
            attn_prologue(0, 0)
            emit_v_unit(0, 0)
            emit_v_unit(0, 1)
            emit_v_unit(0, 2)
        load_bias(2)
        load_bias(3)
        attention(0, 0, [qkh(0, 1, 0), qkh(0, 1, 1), qkh(0, 5, 0),
                         vu(0, 3), vu(0, 4), vu(0, 5), vu(0, 6)],
                  nxt=(0, 1))
        load_wout()
        load_bias(4)
        load_bias(5)
        emit_xt(1)
        attention(0, 1, [qkh(0, 5, 1), qkh(0, 2, 0), qkh(0, 2, 1),
                         qkh(0, 6, 0), vu(1, 0), vu(1, 1), vu(1, 2)],
                  nxt=(0, 2))
        load_bias(6)
        load_bias(7)
        norm_half(0, 0)
        attention(0, 2, [qkh(0, 6, 1), qkh(0, 3, 0), qkh(0, 3, 1),
                         qkh(0, 7, 0), vu(1, 3), vu(1, 4), vu(1, 5)],
                  nxt=(0, 3))
        norm_pair(0, 2)
        attention(0, 3, [qkh(0, 7, 1), qkh(1, 0, 0), qkh(1, 0, 1),
                         qkh(1, 4, 0), vu(1, 6), dmy(), dmy()],
                  nxt=(1, 0))
        norm_pair(0, 3)
        attention(1, 0, [qkh(1, 4, 1), qkh(1, 1, 0), qkh(1, 1, 1),
                         qkh(1, 5, 0), ou(0, 0), ou(0, 1), dmy()],
                  nxt=(1, 1))
        attention(1, 1, [qkh(1, 5, 1), qkh(1, 2, 0), qkh(1, 2, 1),
                         qkh(1, 6, 0), ou(0, 2), ou(0, 3), dmy()],
                  nxt=(1, 2))
        norm_half(1, 0)
        attention(1, 2, [qkh(1, 6, 1), qkh(1, 3, 0), qkh(1, 3, 1),
                         qkh(1, 7, 0), ou(0, 4), ou(0, 5), dmy()],
                  nxt=(1, 3))
        norm_pair(1, 2)
        attention(1, 3, [qkh(1, 7, 1), ou(0, 6), dmy(), dmy(), dmy(), dmy()])
        norm_pair(1, 3)
        emit_dummies(2, "t1")
        with nc.named_scope("proj_b1"):
            for t in range(NT):
                emit_out_unit(1, t, scalar_copy=True)
                emit_dummies(1, f"t2_{t}")

    nc.compile()
    _CACHE['nc'] = nc
    return nc


def host_prep(x, w_qkv, pos_table, w_out):
    x = np.asarray(x, np.float32).reshape(B, N, C)
    wq = np.array(np.asarray(w_qkv, np.float32), copy=True)
    wq[:, :C] *= np.float32(1.0 / np.sqrt(KD))
    wq_bf = wq.astype(ml_dtypes.bfloat16)
    idx = _rel_index()
    biasT = np.ascontiguousarray(np.exp(
        np.asarray(pos_table, np.float32)[:, idx].transpose(0, 2, 1)
    )).astype(ml_dtypes.bfloat16)
    wout = np.ascontiguousarray(np.asarray(w_out, np.float32)).astype(
        ml_dtypes.bfloat16)
    in_maps = []
    for c in range(NCORES):
        xT = np.ascontiguousarray(
            x[c * BPC:(c + 1) * BPC].transpose(0, 2, 1)).astype(
                ml_dtypes.bfloat16)  # [2, 512, 784]
        in_maps.append({"xT": xT, "wqkv": wq_bf, "wout": wout, "biasT": biasT})
    return in_maps


def run(in_maps, trace=False, trace_cores=None):
    import concourse.bass_utils as bass_utils
    nc = build_nc()
    return bass_utils.run_bass_kernel_spmd(
        nc, in_maps, core_ids=list(range(NCORES)),
        trace=trace, trace_cores=trace_cores)


def kernel(x, w_qkv, pos_table, w_out):
    in_maps = host_prep(x, w_qkv, pos_table, w_out)
    res = run(in_maps)
    out = np.stack([np.asarray(r["out"], np.float32) for r in res.results])
    return np.ascontiguousarray(out.reshape(B, HH, WW, C))
